# revision 12
# baseline (speedup 1.0000x reference)
"""Trainium2 Bass kernel for nn_Attention_90074054132266.

Full multi-head attention (B=2, S=4096, D=512, H=8, HD=64) with RoPE on
q/k, sharded over 8 NeuronCores: batch x head-pair (data parallel over
batch, tensor parallel over heads; core c handles batch c//4, heads
2*(c%4), 2*(c%4)+1). Each core computes a partial output projection
(its 2 heads' contribution); the host sums the 4 per-batch partials
(the "all-reduce") and adds wo_b.

Per-core device algorithm (everything stored transposed, f32/f32r):
  - host passes x[b].T, so projections q^T/k^T = wq^T-chunks @ x^T run
    as N=512 f32r matmuls (1 cycle/row).
  - RoPE via duplicated projections with half-swapped weight columns
    (q2^T[d] = q^T[(d+32)%64 per head]) + sign-baked cos/sin tables:
    q_rot = q^T * cosf + q2^T * sinf  (3 VectorE tensor-tensor ops).
  - scores computed transposed: S^T[k-chunk, q] = K_rot slice.T @ Q_rot
    (contraction over d=64; the 2 heads use PE row-groups 0-63/64-127
    concurrently). exp(S/8) runs on ScalarE straight out of PSUM with
    the 1/sqrt(hd) scale folded into the activation (no max
    subtraction: scores ~ N(0,1), exp is safe in fp32).
  - P@V accumulates O'^T[65, q] over the 32 k-chunks where V' has a
    ones column appended: row 64 = softmax denominator Z for free.
  - output projection U_h = O_h^T.T @ wo_h per head (row-group packed),
    normalization by 1/Z_h[q] applied as a per-partition scalar during
    the PSUM evacuation that also sums the two heads.
"""

import sys

sys.path.insert(0, "/opt/trn_rl_repo")

import numpy as np

B, S, DIM, HEADS, HD = 2, 4096, 512, 8, 64
HALF = HD // 2
NCORES = 8
HPC = 2  # heads per core
DPC = HPC * HD  # 128 projection columns per core
NSC = S // 512  # 8 q-column chunks of 512
NKC = S // 128  # 32 k-chunks of 128
NUT = S // 128  # 32 q-row tiles of 128
KC2 = NKC // 2  # 16 pairs of k-chunks (exp batches of [128, 1024])
VW = 2 * (HD + 1)  # 130: per-k V' row for both heads (64+1 each)

_CACHE = {}


def _split_multiwait_drains(nc):
    """The walrus build in this container rejects any instruction with
    more than one sync-wait ("Too many sync wait commands"). Hoist the
    extra waits onto preceding same-engine NoOps, leaving one wait on
    the original instruction."""
    import bass_rust
    import concourse.mybir as mybir

    for fn in nc.m.functions:
        for bb in fn.blocks:
            new_insts = []
            changed = False
            for inst in bb.instructions:
                si = getattr(inst, "sync_info", None)
                if si is not None and len(si.on_wait) > 1:
                    waits = list(si.on_wait)
                    for k, w in enumerate(waits[:-1]):
                        d = mybir.InstNoOp(name=f"{inst.name}w{k}", ins=[], outs=[])
                        d.engine = inst.engine
                        d.sync_info = bass_rust.SyncInfo(on_wait=[w], on_update=[])
                        new_insts.append(d)
                    inst.sync_info = bass_rust.SyncInfo(
                        on_wait=[waits[-1]], on_update=list(si.on_update)
                    )
                    changed = True
                new_insts.append(inst)
            if changed:
                bb.instructions = new_insts


def _build(qk_bias, v_bias):
    import concourse.bass as bass
    import concourse.tile as tile
    from concourse import mybir

    F32 = mybir.dt.float32
    F32R = mybir.dt.float32r
    EXP = mybir.ActivationFunctionType.Exp
    MUL = mybir.AluOpType.mult
    ADD = mybir.AluOpType.add
    SUB = mybir.AluOpType.subtract

    nc = bass.Bass("TRN2")

    xt_e = nc.declare_dram_parameter("xt", [DIM, S], F32, isOutput=False)
    w_e = {}
    for name in ("wq", "wqp", "wk", "wkp", "wv"):
        w_e[name] = nc.declare_dram_parameter(name, [DIM, DPC], F32, isOutput=False)
    wo_e = nc.declare_dram_parameter("wo", [DPC, DIM], F32, isOutput=False)
    cos_e = nc.declare_dram_parameter("cosf", [DPC, S], F32, isOutput=False)
    sin_e = nc.declare_dram_parameter("sinf", [DPC, S], F32, isOutput=False)
    b_e = {}
    if qk_bias:
        for name in ("qb", "qbp", "kb", "kbp"):
            b_e[name] = nc.declare_dram_parameter(name, [DPC, 1], F32, isOutput=False)
    if v_bias:
        b_e["vb"] = nc.declare_dram_parameter("vb", [1, DPC], F32, isOutput=False)
    out_e = nc.declare_dram_parameter("out", [S, DIM], F32, isOutput=True)

    with tile.TileContext(nc) as tc:
        with (
            tc.tile_pool(name="persist", bufs=1) as P,
            tc.tile_pool(name="work", bufs=2) as W,
        ):
            # ---- persistent SBUF tensors ----
            qr = P.tile([DPC, S], F32R, tag="qr")  # rotated q^T
            kr = P.tile([DPC, S], F32R, tag="kr")  # rotated k^T
            # V' rows: per k-chunk st, V[k, :] for head A cols 0:64 + ones
            # col 64, head B cols 65:129 + ones col 129.
            vb_sb = P.tile([128, NKC, VW], F32R, tag="vboth")
            ot = P.tile([DPC, S], F32R, tag="ot")  # unnormalized O^T
            wo_sb = P.tile([DPC, DIM], F32R, tag="wo")
            nc.sync.dma_start(out=wo_sb, in_=wo_e[:].bitcast(F32R))
            zrow = [P.tile([1, S], F32, tag=f"zrow{h}", name=f"zrow{h}") for h in range(HPC)]
            zt = P.tile([128, 2 * NUT], F32, tag="zt")
            izt = P.tile([128, 2 * NUT], F32, tag="izt")

            bias_sb = {}
            if qk_bias:
                for name in ("qb", "qbp", "kb", "kbp"):
                    t = P.tile([DPC, 1], F32, tag=name)
                    nc.sync.dma_start(out=t, in_=b_e[name][:])
                    bias_sb[name] = t
            if v_bias:
                vbias_bc = P.tile([128, DPC], F32, tag="vbias")
                src = bass.AP(
                    tensor=b_e["vb"].tensor,
                    offset=b_e["vb"].offset,
                    ap=[[0, 128], [1, DPC]],
                )
                nc.sync.dma_start(out=vbias_bc, in_=src)

            # ---- phase B+C: projections + rope (xt streamed by chunk) ----
            with (
                tc.tile_pool(name="xtp", bufs=2) as XT,
                tc.tile_pool(name="wpool", bufs=1) as WP,
                tc.tile_pool(name="pps", bufs=2, space="PSUM") as PPS,
            ):
                cos_sb = WP.tile([DPC, S], F32, tag="cos")
                sin_sb = WP.tile([DPC, S], F32, tag="sin")
                nc.sync.dma_start(out=cos_sb, in_=cos_e[:])
                nc.sync.dma_start(out=sin_sb, in_=sin_e[:])
                w_sb = {}
                for name in ("wq", "wqp", "wk", "wkp", "wv"):
                    t = WP.tile([128, 4, DPC], F32R, tag=name)
                    nc.sync.dma_start(
                        out=t, in_=w_e[name][:].rearrange("(c p) m -> p c m", p=128).bitcast(F32R)
                    )
                    w_sb[name] = t

                # ones columns of V' (written once; disjoint from evac cols)
                ones_ap = vb_sb[:].rearrange("p s (j w) -> p s j w", w=HD + 1)[
                    :, :, :, HD : HD + 1
                ]
                nc.vector.memset(ones_ap.bitcast(F32), 1.0)

                xt_r = xt_e[:].rearrange("(c p) s -> c p s", p=128)
                for sc in range(NSC):
                    qs = bass.ts(sc, 512)
                    xt_c = []
                    for c in range(4):
                        t = XT.tile([128, 512], F32R, tag=f"xt{c}", name=f"xt{c}_{sc}")
                        nc.sync.dma_start(out=t, in_=xt_r[c, :, qs].bitcast(F32R))
                        xt_c.append(t)

                    # q/k (+ half-swapped copies) for this 512-col chunk
                    for dst, wn, wpn, bn, bpn in (
                        (qr, "wq", "wqp", "qb", "qbp"),
                        (kr, "wk", "wkp", "kb", "kbp"),
                    ):
                        ps1 = PPS.tile([128, 512], F32, tag="p1")
                        ps2 = PPS.tile([128, 512], F32, tag="p2")
                        for c in range(4):
                            nc.tensor.matmul(
                                ps1,
                                w_sb[wn][:, c, :],
                                xt_c[c][:],
                                start=(c == 0),
                                stop=(c == 3),
                            )
                        for c in range(4):
                            nc.tensor.matmul(
                                ps2,
                                w_sb[wpn][:, c, :],
                                xt_c[c][:],
                                start=(c == 0),
                                stop=(c == 3),
                            )
                        if qk_bias:
                            t1 = W.tile([128, 512], F32, tag="rope1")
                            t2 = W.tile([128, 512], F32, tag="rope2")
                            nc.vector.tensor_scalar_add(t1, ps1, bias_sb[bn])
                            nc.vector.tensor_scalar_add(t2, ps2, bias_sb[bpn])
                            s1, s2 = t1, t2
                        else:
                            s1, s2 = ps1, ps2
                        t3 = W.tile([128, 512], F32, tag="rope3")
                        t4 = W.tile([128, 512], F32, tag="rope4")
                        nc.vector.tensor_tensor(
                            out=t3, in0=s1, in1=cos_sb[:, qs], op=MUL
                        )
                        nc.vector.tensor_tensor(
                            out=t4, in0=s2, in1=sin_sb[:, qs], op=MUL
                        )
                        nc.vector.tensor_tensor(
                            out=dst[:, qs], in0=t3, in1=t4, op=ADD
                        )

                    # V projection rows for the 4 k-chunks inside this chunk
                    for stl in range(4):
                        st = sc * 4 + stl
                        psv = PPS.tile([128, 128], F32, tag="pv")
                        for c in range(4):
                            nc.tensor.matmul(
                                psv,
                                xt_c[c][:, bass.ts(stl, 128)],
                                w_sb["wv"][:, c, :],
                                start=(c == 0),
                                stop=(c == 3),
                            )
                        dsts = vb_sb[:, st, :].rearrange(
                            "p (j w) -> p j w", w=HD + 1
                        )[:, :, 0:HD]
                        if v_bias:
                            nc.vector.tensor_tensor(
                                out=dsts, in0=psv, in1=vbias_bc, op=ADD
                            )
                        else:
                            nc.vector.tensor_copy(out=dsts, in_=psv)

            # ---- phase D: attention ----
            with (
                tc.tile_pool(name="pss", bufs=2, space="PSUM") as PSS,
                tc.tile_pool(name="pso", bufs=2, space="PSUM") as PSO,
            ):
                for h in range(HPC):
                    hs = slice(h * HD, (h + 1) * HD)
                    vcol = slice(h * (HD + 1), (h + 1) * (HD + 1))
                    for qt in range(NSC):
                        qs = bass.ts(qt, 512)
                        pso_t = PSO.tile([HD + 1, 512], F32, tag="o")
                        for kc2 in range(KC2):
                            pss_t = PSS.tile([128, 1024], F32, tag="s")
                            for j in range(2):
                                kc = kc2 * 2 + j
                                nc.tensor.matmul(
                                    pss_t[:, bass.ts(j, 512)],
                                    kr[hs, bass.ts(kc, 128)],
                                    qr[hs, qs],
                                    start=True,
                                    stop=True,
                                )
                            pt = W.tile([128, 1024], F32R, tag="pt")
                            nc.scalar.activation(
                                out=pt, in_=pss_t, func=EXP, scale=0.125
                            )
                            for j in range(2):
                                kc = kc2 * 2 + j
                                nc.tensor.matmul(
                                    pso_t,
                                    vb_sb[:, kc, vcol],
                                    pt[:, bass.ts(j, 512)],
                                    start=(kc == 0),
                                    stop=(kc == NKC - 1),
                                )
                        nc.vector.tensor_copy(out=ot[hs, qs], in_=pso_t[0:HD, :])
                        nc.vector.tensor_copy(
                            out=zrow[h][0:1, qs], in_=pso_t[HD : HD + 1, :]
                        )

            # ---- phase E: Z transpose (via DRAM bounce) + reciprocal ----
            zs = nc.dram_tensor("zscratch", [HPC, S], F32)
            for h in range(HPC):
                nc.sync.dma_start(out=zs[h : h + 1, :], in_=zrow[h][:])
            for h in range(HPC):
                nc.sync.dma_start(
                    out=zt[:, bass.ts(h, NUT)],
                    in_=zs[h, :].rearrange("(j p) -> p j", p=128),
                )
            nc.vector.reciprocal(out=izt, in_=zt)

            # ---- phase F: output projection + normalize + head sum ----
            with tc.tile_pool(name="psu", bufs=4, space="PSUM") as PSU:
                for ut in range(NUT):
                    us = bass.ts(ut, 128)
                    psu = [PSU.tile([128, DIM], F32, tag=f"u{h}", name=f"u{h}_{ut}") for h in range(HPC)]
                    for h in range(HPC):
                        hs = slice(h * HD, (h + 1) * HD)
                        nc.tensor.matmul(
                            psu[h],
                            ot[hs, us],
                            wo_sb[hs, :],
                            start=True,
                            stop=True,
                        )
                    t_mid = W.tile([128, DIM], F32, tag="umid")
                    nc.vector.tensor_scalar_mul(t_mid, psu[0], izt[:, ut : ut + 1])
                    t_out = W.tile([128, DIM], F32, tag="uout")
                    nc.vector.scalar_tensor_tensor(
                        out=t_out,
                        in0=psu[1],
                        scalar=izt[:, NUT + ut : NUT + ut + 1],
                        in1=t_mid,
                        op0=MUL,
                        op1=ADD,
                    )
                    nc.sync.dma_start(out=out_e[us, :], in_=t_out)

    return nc


def _rope_tables():
    freqs = 10000.0 ** (-np.linspace(0.0, 1.0, HALF, endpoint=False))
    theta = np.arange(S, dtype=np.float64)[None, :] * freqs[:, None]  # [32, S]
    cos32 = np.cos(theta)
    sin32 = np.sin(theta)
    cosf = np.tile(np.concatenate([cos32, cos32], axis=0), (HPC, 1))
    sinf = np.tile(np.concatenate([-sin32, sin32], axis=0), (HPC, 1))
    return cosf.astype(np.float32), sinf.astype(np.float32)


def kernel(x, wq_k, wq_b, wk_k, wk_b, wv_k, wv_b, wo_k, wo_b):
    from concourse.bass_utils import run_bass_kernel_spmd

    x = np.asarray(x, np.float32)
    wq_k = np.asarray(wq_k, np.float32)
    wq_b = np.asarray(wq_b, np.float32)
    wk_k = np.asarray(wk_k, np.float32)
    wk_b = np.asarray(wk_b, np.float32)
    wv_k = np.asarray(wv_k, np.float32)
    wv_b = np.asarray(wv_b, np.float32)
    wo_k = np.asarray(wo_k, np.float32)
    wo_b = np.asarray(wo_b, np.float32)

    qk_bias = bool(np.any(wq_b) or np.any(wk_b))
    v_bias = bool(np.any(wv_b))

    key = (qk_bias, v_bias)
    if key not in _CACHE:
        nc = _build(qk_bias, v_bias)
        _split_multiwait_drains(nc)
        _CACHE[key] = nc
    nc = _CACHE[key]

    cosf, sinf = _rope_tables()
    perm = np.r_[HALF:HD, 0:HALF]

    in_maps = []
    for c in range(NCORES):
        b = c // 4
        h0 = HPC * (c % 4)
        hsl = slice(h0, h0 + HPC)
        m = {
            "xt": np.ascontiguousarray(x[b].T),
            "wq": np.ascontiguousarray(wq_k[:, hsl, :].reshape(DIM, DPC)),
            "wqp": np.ascontiguousarray(wq_k[:, hsl, perm].reshape(DIM, DPC)),
            "wk": np.ascontiguousarray(wk_k[:, hsl, :].reshape(DIM, DPC)),
            "wkp": np.ascontiguousarray(wk_k[:, hsl, perm].reshape(DIM, DPC)),
            "wv": np.ascontiguousarray(wv_k[:, hsl, :].reshape(DIM, DPC)),
            "wo": np.ascontiguousarray(wo_k[hsl].reshape(DPC, DIM)),
            "cosf": cosf,
            "sinf": sinf,
        }
        if qk_bias:
            m["qb"] = np.ascontiguousarray(wq_b[hsl].reshape(DPC, 1))
            m["qbp"] = np.ascontiguousarray(wq_b[hsl][:, perm].reshape(DPC, 1))
            m["kb"] = np.ascontiguousarray(wk_b[hsl].reshape(DPC, 1))
            m["kbp"] = np.ascontiguousarray(wk_b[hsl][:, perm].reshape(DPC, 1))
        if v_bias:
            m["vb"] = np.ascontiguousarray(wv_b[hsl].reshape(1, DPC))
        in_maps.append(m)

    res = run_bass_kernel_spmd(nc, in_maps, list(range(NCORES)))

    out = np.zeros((B, S, DIM), np.float32)
    for c in range(NCORES):
        out[c // 4] += res.results[c]["out"]
    out += wo_b[None, None, :]
    return out
